# revision 8
# baseline (speedup 1.0000x reference)
"""BlockDiffusionDecoder Trainium2 kernel.

Data-parallel over batch (8 batches/core x 8 cores). Feature-major
activations; fp32r matmuls; k-major attention with PE-computed softmax
denominators.

kernel(**inputs) takes FULL unsharded inputs (keyed as in
reference.setup_inputs) and returns the FULL (64, 256, 1024) output.
"""
import numpy as np

import concourse.bass as bass
import concourse.mybir as mybir
import concourse.tile as tile
from concourse import bacc
from concourse.bass_utils import run_bass_kernel_spmd

# problem dims (hardcoded per harness contract)
B, S, D, H, CD, TV = 64, 128, 1024, 16, 128, 512
HD = D // H            # 64
MLP = 4 * D            # 4096
T = 2 * S              # 256 tokens per batch
NCORES = 8
BL = B // NCORES       # 8 batches per core
NPAIR = BL // 2        # batch pairs for wide GEMM stages
KC = D // 128          # 8 feature chunks
MC = MLP // 128        # 32 mlp chunks

F32 = mybir.dt.float32
F32R = mybir.dt.float32r
AF = mybir.ActivationFunctionType
ALU = mybir.AluOpType

NEG = -30000.0         # additive mask for "don't attend"
EXP_SHIFT = -8.0       # constant inside exp for range safety


# ---------------------------------------------------------------------------
# device program
# ---------------------------------------------------------------------------

def build_nc(debug=False):
    nc = bacc.Bacc()
    dbg = "ExternalOutput" if debug else "Internal"

    def param(name, shape, dt=F32R):
        return nc.declare_dram_parameter(name, list(shape), dt, isOutput=False)

    # per-core activations
    xT = param("xT", (BL, D, T))                 # x^T feature-major
    cT = param("cT", (CD, BL))
    encT = param("encT", (BL, D, TV))
    encb = param("encb", (128, BL, 4), F32)      # additive enc-mask cols (w/ shift)

    # weights, packed partition-first: [128, out_chunk, k_chunk, 128]
    wqk = param("wqk", (128, 16, KC, 128))       # q,k cols of w_qkv (q /8)
    wv = param("wv", (128, KC, 2, 512))          # v cols of w_qkv (rhs layout)
    wada = param("wada", (128, 48, 128))
    wao = param("wao", (128, KC, KC, 128))
    wqca = param("wqca", (128, KC, KC, 128))     # w_q / 8
    wkca = param("wkca", (128, KC, KC, 128))     # k cols of w_kv
    wvca = param("wvca", (128, KC, 2, 512))      # v cols of w_kv (rhs layout)
    woca = param("woca", (128, KC, KC, 128))
    wm1 = param("wm1", (128, MC, KC, 128))
    wm2 = param("wm2", (128, KC, MC, 128))

    # constants
    rmat = param("rmat", (128, 128))             # R^T for rope rotate
    iden = param("iden", (128, 128))
    onec = param("onec", (128, 1))               # ones col  [K=128, M=1]
    oner = param("oner", (1, 128))               # ones row  [K=1, M<=128]
    maskT = param("maskT", (128, 2, T))          # additive self mask^T tiles
    cosT = param("cosT", (128, 512), F32)
    sinT = param("sinT", (128, 512), F32)
    vecs = param("vecs", (128, KC, 3), F32)      # norm1_w / ca_norm_w / norm2_w
    b1c = param("b1c", (128, MC), F32)
    b2c = param("b2c", (128, KC), F32)
    abc = param("abc", (128, 48), F32)

    outT = nc.declare_dram_parameter("outT", [BL, D, T], F32, isOutput=True)

    # scratch
    qT = nc.dram_tensor("qT", [NPAIR, KC, 128, 512], F32R, kind=dbg)
    kT = nc.dram_tensor("kT", [NPAIR, KC, 128, 512], F32R, kind=dbg)
    vtk = nc.dram_tensor("vtk", [BL, 2, 128, D], F32R, kind=dbg)
    x2T = nc.dram_tensor("x2T", [BL, KC, 128, T], F32R, kind=dbg)
    kcT = nc.dram_tensor("kcT", [BL, KC, 128, TV], F32R, kind=dbg)
    vcT = nc.dram_tensor("vcT", [BL, 4, 128, D], F32R, kind=dbg)
    x3T = nc.dram_tensor("x3T", [BL, KC, 128, T], F32R, kind=dbg)
    gT = nc.dram_tensor("gT", [NPAIR, MC, 128, 512], F32R, kind=dbg)

    with tile.TileContext(nc) as tc:
        # ------------------------------------------------------- persistent
        pers = tc.alloc_tile_pool(name="pers", bufs=1)
        rm_sb = pers.tile_from(rmat[:])
        id_sb = pers.tile_from(iden[:])
        onec_sb = pers.tile_from(onec[:])
        oner_sb = pers.tile_from(oner[:])
        mskT_sb = pers.tile_from(maskT[:])
        cos_sb = pers.tile_from(cosT[:])
        sin_sb = pers.tile_from(sinT[:])
        vec_sb = pers.tile_from(vecs[:])
        b1_sb = pers.tile_from(b1c[:])
        b2_sb = pers.tile_from(b2c[:])
        ab_sb = pers.tile_from(abc[:])
        enb_sb = pers.tile_from(encb[:])
        ct_sb = pers.tile_from(cT[:])

        # mod = c @ adaLN_w + b  (feature-major cols [128, BL] per chunk f)
        esh_sb = pers.tile([128, 1], F32)
        nc.vector.memset(esh_sb[:], EXP_SHIFT)
        mod_sb = pers.tile([128, 48, BL], F32)
        sc1_sb = pers.tile([128, KC, BL], F32)   # (1+sc_msa)*norm1_w
        sc2_sb = pers.tile([128, KC, BL], F32)   # (1+sc_mlp)*norm2_w
        b2g_sb = pers.tile([128, KC, BL], F32)   # b_mlp2 * g_mlp
        with tc.psum_pool(name="modps", bufs=1) as mps, \
             tc.sbuf_pool(name="modsb", bufs=1) as msb:
            wada_sb = msb.tile_from(wada[:])
            for f in range(48):
                mp = mps.tile([128, BL], F32, tag="mp", bufs=2)
                nc.tensor.matmul(mp[:], wada_sb[:, f, :], ct_sb[:],
                                 start=True, stop=True)
                nc.scalar.activation(mod_sb[:, f, :], mp[:], AF.Identity,
                                     bias=ab_sb[:, f:f + 1], scale=1.0)
            for c in range(KC):
                nc.vector.tensor_scalar(sc1_sb[:, c, :], mod_sb[:, 8 + c, :],
                                        1.0, vec_sb[:, c, 0:1],
                                        ALU.add, ALU.mult)
                nc.vector.tensor_scalar(sc2_sb[:, c, :], mod_sb[:, 32 + c, :],
                                        1.0, vec_sb[:, c, 2:3],
                                        ALU.add, ALU.mult)
                nc.vector.tensor_scalar(b2g_sb[:, c, :], mod_sb[:, 40 + c, :],
                                        b2_sb[:, c:c + 1], None, ALU.mult)

        def mcol(base, c, b):
            # modulate column [128,1]: base chunk index offset
            return mod_sb[:, base + c, b:b + 1]

        # feature-major LN: src(c)/out(c) -> [128, W] APs.
        # scale_fn/shift_fn(c, half_index) -> [128,1] col or float
        def layer_norm(sbp, psp, W, src, out, scale_fn, shift_fn, halves):
            sum_ps = psp.tile([1, W], F32, tag="lnrow", bufs=2)
            ssq_ps = psp.tile([1, W], F32, tag="lnrow", bufs=2)
            for c in range(KC):
                sq = sbp.tile([128, W], F32R, tag="lnsq", bufs=2)
                nc.scalar.square(sq[:], src(c))
                nc.tensor.matmul(sum_ps[:], onec_sb[:], src(c),
                                 start=(c == 0), stop=(c == KC - 1))
                nc.tensor.matmul(ssq_ps[:], onec_sb[:], sq[:],
                                 start=(c == 0), stop=(c == KC - 1))
            mean = sbp.tile([1, W], F32R, tag="lnmean", bufs=1)
            nc.vector.tensor_scalar(mean[:], sum_ps[:], 1.0 / D, None, ALU.mult)
            ex2 = sbp.tile([1, W], F32, tag="lnex2", bufs=1)
            nc.vector.tensor_scalar(ex2[:], ssq_ps[:], 1.0 / D, None, ALU.mult)
            var = sbp.tile([1, W], F32, tag="lnvar", bufs=1)
            nc.vector.tensor_tensor(var[:], mean[:], mean[:], ALU.mult)
            nc.vector.tensor_tensor(var[:], ex2[:], var[:], ALU.subtract)
            nc.vector.tensor_scalar(var[:], var[:], 1e-5, None, ALU.add)
            std = sbp.tile([1, W], F32, tag="lnstd", bufs=1)
            nc.scalar.activation(std[:], var[:], AF.Sqrt, bias=0.0, scale=1.0)
            rs = sbp.tile([1, W], F32R, tag="lnrs", bufs=1)
            with nc.allow_low_precision(reason="fp32r rounding ok"):
                nc.vector.reciprocal(rs[:], std[:])
            mb_ps = psp.tile([128, W], F32, tag="lnbc", bufs=2)
            rs_ps = psp.tile([128, W], F32, tag="lnbc", bufs=2)
            nc.tensor.matmul(mb_ps[:], oner_sb[:], mean[:], start=True, stop=True)
            nc.tensor.matmul(rs_ps[:], oner_sb[:], rs[:], start=True, stop=True)
            for c in range(KC):
                t = sbp.tile([128, W], F32, tag="lnt", bufs=2)
                nc.vector.tensor_tensor(t[:], src(c), mb_ps[:], ALU.subtract)
                t2 = sbp.tile([128, W], F32, tag="lnt2", bufs=2)
                nc.vector.tensor_tensor(t2[:], t[:], rs_ps[:], ALU.mult)
                o = out(c)
                for hi, (lo, hi_) in enumerate(halves):
                    nc.scalar.activation(
                        o[:, lo:hi_], t2[:, lo:hi_], AF.Identity,
                        bias=shift_fn(c, hi), scale=scale_fn(c, hi))

        # ------------------------------------------------- stage 1: LN1+QKV
        with tc.sbuf_pool(name="s1w", bufs=1) as s1w, \
             tc.sbuf_pool(name="s1a", bufs=1) as s1a, \
             tc.sbuf_pool(name="s1t", bufs=1) as s1t, \
             tc.psum_pool(name="s1p", bufs=1) as s1p:
            wqk_sb = s1w.tile_from(wqk[:])
            wv_sb = s1w.tile_from(wv[:])
            for p in range(NPAIR):
                b0, b1 = 2 * p, 2 * p + 1
                x_sb = s1a.tile([128, KC, 512], F32R, tag="x", bufs=1)
                for c in range(KC):
                    nc.sync.dma_start(x_sb[:, c, 0:256],
                                      xT[b0, 128 * c:128 * (c + 1), :])
                    nc.sync.dma_start(x_sb[:, c, 256:512],
                                      xT[b1, 128 * c:128 * (c + 1), :])
                xn_sb = s1a.tile([128, KC, 512], F32R, tag="xn", bufs=1)
                layer_norm(
                    s1t, s1p, 512,
                    src=lambda c: x_sb[:, c, :],
                    out=lambda c: xn_sb[:, c, :],
                    scale_fn=lambda c, hi: sc1_sb[:, c, b0 + hi:b0 + hi + 1],
                    shift_fn=lambda c, hi: mcol(0, c, b0 + hi),
                    halves=[(0, 256), (256, 512)])

                # q, k feature-major chunks + rope
                for f in range(16):
                    dst = qT if f < 8 else kT
                    qp = s1p.tile([128, 512], F32, tag="gemm", bufs=2)
                    for c in range(KC):
                        nc.tensor.matmul(qp[:], wqk_sb[:, f, c, :], xn_sb[:, c, :],
                                         start=(c == 0), stop=(c == KC - 1))
                    q_sb = s1t.tile([128, 512], F32R, tag="q", bufs=2)
                    nc.scalar.copy(q_sb[:], qp[:])
                    rp = s1p.tile([128, 512], F32, tag="rot", bufs=2)
                    nc.tensor.matmul(rp[:], rm_sb[:], q_sb[:], start=True, stop=True)
                    m1 = s1t.tile([128, 512], F32, tag="m1", bufs=2)
                    nc.vector.tensor_tensor(m1[:], q_sb[:], cos_sb[:], ALU.mult)
                    qr = s1t.tile([128, 512], F32R, tag="qr", bufs=2)
                    nc.vector.tensor_tensor(qr[:], rp[:], sin_sb[:], ALU.mult)
                    nc.vector.tensor_tensor(qr[:], m1[:], qr[:], ALU.add)
                    nc.sync.dma_start(dst[p, f % 8], qr[:])

                # v token-major
                for hi in range(2):
                    for tt in range(2):
                        for fc in range(2):
                            vp = s1p.tile([128, 512], F32, tag="gemm", bufs=2)
                            lo = 256 * hi + 128 * tt
                            for c in range(KC):
                                nc.tensor.matmul(vp[:], xn_sb[:, c, lo:lo + 128],
                                                 wv_sb[:, c, fc, :],
                                                 start=(c == 0), stop=(c == KC - 1))
                            v_sb = s1t.tile([128, 512], F32R, tag="v", bufs=2)
                            nc.scalar.copy(v_sb[:], vp[:])
                            nc.sync.dma_start(
                                vtk[2 * p + hi, tt, :, 512 * fc:512 * (fc + 1)],
                                v_sb[:])

        # --------------------------------------- stage 2: self-attn + proj
        with tc.sbuf_pool(name="s2w", bufs=1) as s2w, \
             tc.sbuf_pool(name="s2a", bufs=1) as s2a, \
             tc.sbuf_pool(name="s2t", bufs=1) as s2t, \
             tc.psum_pool(name="s2p", bufs=1) as s2p:
            wao_sb = s2w.tile_from(wao[:])
            for b in range(BL):
                p, hf = b // 2, b % 2
                qb = s2a.tile([128, KC, 256], F32R, tag="qb", bufs=2)
                kb = s2a.tile([128, KC, 256], F32R, tag="kb", bufs=2)
                vb = s2a.tile([128, 2, D], F32R, tag="vb", bufs=2)
                xb = s2a.tile([128, KC, 256], F32R, tag="xb", bufs=2)
                for c in range(KC):
                    nc.sync.dma_start(qb[:, c, :], qT[p, c, :, 256 * hf:256 * (hf + 1)])
                    nc.sync.dma_start(kb[:, c, :], kT[p, c, :, 256 * hf:256 * (hf + 1)])
                    nc.sync.dma_start(xb[:, c, :], xT[b, 128 * c:128 * (c + 1), :])
                for kt in range(2):
                    nc.sync.dma_start(vb[:, kt, :], vtk[b, kt])
                a_sb = s2a.tile([128, KC, 256], F32R, tag="ab", bufs=2)
                for h in range(H):
                    hr = slice(64 * (h % 2), 64 * (h % 2) + 64)
                    fc = h // 2
                    pt_sb = s2t.tile([128, 2, 256], F32R, tag="pt", bufs=2)
                    dn_ps = s2p.tile([1, 256], F32, tag="dn", bufs=1)
                    for kt in range(2):
                        sp = s2p.tile([128, 256], F32, tag="sp", bufs=2)
                        nc.tensor.matmul(sp[:], kb[hr, fc, 128 * kt:128 * (kt + 1)],
                                         qb[hr, fc, :], start=True, stop=False)
                        nc.tensor.matmul(sp[:], id_sb[:], mskT_sb[:, kt, :],
                                         start=False, stop=True)
                        nc.scalar.activation(pt_sb[:, kt, :], sp[:], AF.Exp,
                                             bias=esh_sb[:], scale=1.0)
                        nc.tensor.matmul(dn_ps[:], onec_sb[:], pt_sb[:, kt, :],
                                         start=(kt == 0), stop=(kt == 1))
                    rc_sb = s2t.tile([1, 256], F32R, tag="rc", bufs=2)
                    with nc.allow_low_precision(reason="fp32r rounding ok"):
                        nc.vector.reciprocal(rc_sb[:], dn_ps[:])
                    rb_ps = s2p.tile([64, 256], F32, tag="rb", bufs=1)
                    nc.tensor.matmul(rb_ps[:], oner_sb[0:1, 0:64], rc_sb[:],
                                     start=True, stop=True)
                    rb_sb = s2t.tile([64, 256], F32, tag="rbs", bufs=2)
                    nc.scalar.copy(rb_sb[:], rb_ps[:])
                    av_ps = s2p.tile([64, 256], F32, tag="av", bufs=2)
                    for kt in range(2):
                        nc.tensor.matmul(av_ps[:], vb[:, kt, 64 * h:64 * (h + 1)],
                                         pt_sb[:, kt, :],
                                         start=(kt == 0), stop=(kt == 1))
                    nc.vector.tensor_tensor(a_sb[hr, fc, :], av_ps[:], rb_sb[:],
                                            ALU.mult)
                for oc in range(KC):
                    op = s2p.tile([128, 256], F32, tag="op", bufs=2)
                    for c in range(KC):
                        nc.tensor.matmul(op[:], wao_sb[:, oc, c, :], a_sb[:, c, :],
                                         start=(c == 0), stop=(c == KC - 1))
                    x2_sb = s2t.tile([128, 256], F32R, tag="x2", bufs=2)
                    nc.vector.scalar_tensor_tensor(x2_sb[:], op[:],
                                                   mcol(16, oc, b), xb[:, oc, :],
                                                   ALU.mult, ALU.add)
                    nc.sync.dma_start(x2T[b, oc], x2_sb[:])

        # ------------------------------------------------ stage 3: CA k/v
        with tc.sbuf_pool(name="s3w", bufs=1) as s3w, \
             tc.sbuf_pool(name="s3a", bufs=1) as s3a, \
             tc.sbuf_pool(name="s3t", bufs=1) as s3t, \
             tc.psum_pool(name="s3p", bufs=1) as s3p:
            wk_sb = s3w.tile_from(wkca[:])
            wvc_sb = s3w.tile_from(wvca[:])
            for b in range(BL):
                e_sb = s3a.tile([128, KC, TV], F32R, tag="e", bufs=2)
                for c in range(KC):
                    nc.sync.dma_start(e_sb[:, c, :], encT[b, 128 * c:128 * (c + 1), :])
                for f in range(KC):
                    kp = s3p.tile([128, TV], F32, tag="gemm", bufs=2)
                    for c in range(KC):
                        nc.tensor.matmul(kp[:], wk_sb[:, f, c, :], e_sb[:, c, :],
                                         start=(c == 0), stop=(c == KC - 1))
                    kc_sb = s3t.tile([128, TV], F32R, tag="kc", bufs=2)
                    nc.scalar.copy(kc_sb[:], kp[:])
                    nc.sync.dma_start(kcT[b, f], kc_sb[:])
                for tt in range(4):
                    for fcc in range(2):
                        vp = s3p.tile([128, 512], F32, tag="gemm", bufs=2)
                        for c in range(KC):
                            nc.tensor.matmul(vp[:], e_sb[:, c, 128 * tt:128 * (tt + 1)],
                                             wvc_sb[:, c, fcc, :],
                                             start=(c == 0), stop=(c == KC - 1))
                        vc_sb = s3t.tile([128, 512], F32R, tag="vc", bufs=2)
                        nc.scalar.copy(vc_sb[:], vp[:])
                        nc.sync.dma_start(vcT[b, tt, :, 512 * fcc:512 * (fcc + 1)],
                                          vc_sb[:])

        # ------------------------------------- stage 4: CA attn + proj
        with tc.sbuf_pool(name="s4w", bufs=1) as s4w, \
             tc.sbuf_pool(name="s4a", bufs=1) as s4a, \
             tc.sbuf_pool(name="s4t", bufs=1) as s4t:
            wq_sb = s4w.tile_from(wqca[:])
            wo_sb = s4w.tile_from(woca[:])
            for b in range(BL):
                x2_sb = s4a.tile([128, KC, 256], F32R, tag="x2b", bufs=1)
                for c in range(KC):
                    nc.sync.dma_start(x2_sb[:, c, :], x2T[b, c])
                kc_sb = s4a.tile([128, KC, TV], F32R, tag="kcb", bufs=2)
                vc_sb = s4a.tile([128, 4, D], F32R, tag="vcb", bufs=2)
                for c in range(KC):
                    nc.sync.dma_start(kc_sb[:, c, :], kcT[b, c])
                for kt in range(4):
                    nc.sync.dma_start(vc_sb[:, kt, :], vcT[b, kt])
                xc_sb = s4a.tile([128, KC, 256], F32R, tag="xc", bufs=1)
                qc_sb = s4a.tile([128, KC, 256], F32R, tag="qc", bufs=1)
                with tc.psum_pool(name="s4pl", bufs=1) as s4pl:
                    layer_norm(
                        s4t, s4pl, 256,
                        src=lambda c: x2_sb[:, c, :],
                        out=lambda c: xc_sb[:, c, :],
                        scale_fn=lambda c, hi: vec_sb[:, c, 1:2],
                        shift_fn=lambda c, hi: 0.0,
                        halves=[(0, 256)])
                    for f in range(KC):
                        qp = s4pl.tile([128, 256], F32, tag="lnbc", bufs=2)
                        for c in range(KC):
                            nc.tensor.matmul(qp[:], wq_sb[:, f, c, :], xc_sb[:, c, :],
                                             start=(c == 0), stop=(c == KC - 1))
                        nc.scalar.copy(qc_sb[:, f, :], qp[:])
                ac_sb = s4a.tile([128, KC, 256], F32R, tag="ac", bufs=1)
                with tc.psum_pool(name="s4pa", bufs=1) as s4pa:
                    for h in range(H):
                        hr = slice(64 * (h % 2), 64 * (h % 2) + 64)
                        fc = h // 2
                        pt_sb = s4t.tile([128, 4, 256], F32R, tag="cpt", bufs=2)
                        dn_ps = s4pa.tile([1, 256], F32, tag="cdn", bufs=1)
                        for kt in range(4):
                            sp = s4pa.tile([128, 256], F32, tag="csp", bufs=2)
                            nc.tensor.matmul(sp[:],
                                             kc_sb[hr, fc, 128 * kt:128 * (kt + 1)],
                                             qc_sb[hr, fc, :], start=True, stop=True)
                            nc.scalar.activation(pt_sb[:, kt, :], sp[:], AF.Exp,
                                                 bias=enb_sb[:, b, kt:kt + 1],
                                                 scale=1.0)
                            nc.tensor.matmul(dn_ps[:], onec_sb[:], pt_sb[:, kt, :],
                                             start=(kt == 0), stop=(kt == 3))
                        rc_sb = s4t.tile([1, 256], F32R, tag="crc", bufs=2)
                        with nc.allow_low_precision(reason="fp32r rounding ok"):
                            nc.vector.reciprocal(rc_sb[:], dn_ps[:])
                        rb_ps = s4pa.tile([64, 256], F32, tag="crb", bufs=1)
                        nc.tensor.matmul(rb_ps[:], oner_sb[0:1, 0:64], rc_sb[:],
                                         start=True, stop=True)
                        rb_sb = s4t.tile([64, 256], F32, tag="crbs", bufs=2)
                        nc.scalar.copy(rb_sb[:], rb_ps[:])
                        av_ps = s4pa.tile([64, 256], F32, tag="cav", bufs=2)
                        for kt in range(4):
                            nc.tensor.matmul(av_ps[:],
                                             vc_sb[:, kt, 64 * h:64 * (h + 1)],
                                             pt_sb[:, kt, :],
                                             start=(kt == 0), stop=(kt == 3))
                        nc.vector.tensor_tensor(ac_sb[hr, fc, :], av_ps[:],
                                                rb_sb[:], ALU.mult)
                    for oc in range(KC):
                        op = s4pa.tile([128, 256], F32, tag="cop", bufs=2)
                        for c in range(KC):
                            nc.tensor.matmul(op[:], wo_sb[:, oc, c, :],
                                             ac_sb[:, c, :],
                                             start=(c == 0), stop=(c == KC - 1))
                        x3_sb = s4t.tile([128, 256], F32R, tag="x3", bufs=2)
                        nc.vector.tensor_tensor(x3_sb[:], op[:], x2_sb[:, oc, :],
                                                ALU.add)
                        nc.sync.dma_start(x3T[b, oc], x3_sb[:])

        # ---------------------------------------------- stage 5: LN2+MLP1
        with tc.sbuf_pool(name="s5h", bufs=1) as s5h, \
             tc.sbuf_pool(name="s5w", bufs=1) as s5w, \
             tc.sbuf_pool(name="s5t", bufs=1) as s5t, \
             tc.psum_pool(name="s5p", bufs=1) as s5p:
            h_sb = s5h.tile([128, NPAIR, KC, 512], F32R)
            for p in range(NPAIR):
                b0 = 2 * p
                x3_sb = s5t.tile([128, KC, 512], F32R, tag="x3b", bufs=2)
                for c in range(KC):
                    nc.sync.dma_start(x3_sb[:, c, 0:256], x3T[b0, c])
                    nc.sync.dma_start(x3_sb[:, c, 256:512], x3T[b0 + 1, c])
                layer_norm(
                    s5t, s5p, 512,
                    src=lambda c: x3_sb[:, c, :],
                    out=lambda c: h_sb[:, p, c, :],
                    scale_fn=lambda c, hi: sc2_sb[:, c, b0 + hi:b0 + hi + 1],
                    shift_fn=lambda c, hi: mcol(24, c, b0 + hi),
                    halves=[(0, 256), (256, 512)])
            for mc in range(MC):
                wt = s5w.tile([128, KC, 128], F32R, tag="w1", bufs=3)
                nc.sync.dma_start(wt[:], wm1[:, mc])
                for p in range(NPAIR):
                    gp = s5p.tile([128, 512], F32, tag="lnbc", bufs=2)
                    for c in range(KC):
                        nc.tensor.matmul(gp[:], wt[:, c, :], h_sb[:, p, c, :],
                                         start=(c == 0), stop=(c == KC - 1))
                    g_sb = s5t.tile([128, 512], F32R, tag="g", bufs=3)
                    nc.scalar.activation(g_sb[:], gp[:], AF.Gelu_apprx_tanh,
                                         bias=b1_sb[:, mc:mc + 1], scale=1.0)
                    nc.sync.dma_start(gT[p, mc], g_sb[:])

        # ------------------------------------- stage 6: MLP2 + residual
        with tc.sbuf_pool(name="s6w", bufs=1) as s6w, \
             tc.sbuf_pool(name="s6a", bufs=1) as s6a, \
             tc.sbuf_pool(name="s6t", bufs=1) as s6t, \
             tc.psum_pool(name="s6p", bufs=1) as s6p:
            wm2_sb = s6w.tile_from(wm2[:])
            for p in range(NPAIR):
                b0 = 2 * p
                x3_sb = s6a.tile([128, KC, 512], F32R, tag="x3c", bufs=2)
                for c in range(KC):
                    nc.sync.dma_start(x3_sb[:, c, 0:256], x3T[b0, c])
                    nc.sync.dma_start(x3_sb[:, c, 256:512], x3T[b0 + 1, c])
                mp = [s6p.tile([128, 512], F32, tag=f"mp{oc}", bufs=1, name=f"mp{oc}")
                      for oc in range(KC)]
                for kcc in range(MC):
                    g_sb = s6a.tile([128, 512], F32R, tag="gin", bufs=3)
                    nc.sync.dma_start(g_sb[:], gT[p, kcc])
                    for oc in range(KC):
                        nc.tensor.matmul(mp[oc][:], wm2_sb[:, oc, kcc, :], g_sb[:],
                                         start=(kcc == 0), stop=(kcc == MC - 1))
                for oc in range(KC):
                    for hi in range(2):
                        bb = b0 + hi
                        sl = slice(256 * hi, 256 * (hi + 1))
                        gm = s6t.tile([128, 256], F32, tag="gm", bufs=2)
                        nc.scalar.activation(gm[:], mp[oc][:, sl], AF.Identity,
                                             bias=b2g_sb[:, oc, bb:bb + 1],
                                             scale=mcol(40, oc, bb))
                        o_sb = s6t.tile([128, 256], F32, tag="o", bufs=2)
                        nc.vector.tensor_tensor(o_sb[:], gm[:], x3_sb[:, oc, sl],
                                                ALU.add)
                        nc.sync.dma_start(outT[bb, 128 * oc:128 * (oc + 1), :],
                                          o_sb[:])

        pers.release()

    nc.compile()
    return nc


# ---------------------------------------------------------------------------
# host side
# ---------------------------------------------------------------------------

def _pack_lhsT(w, n_out_chunks, n_k_chunks):
    # w: (K, M) -> (128, out_chunk, k_chunk, 128);
    # [:, oc, kc, :] = w[kc*128:(kc+1)*128, oc*128:(oc+1)*128]
    K, M = w.shape
    assert K == n_k_chunks * 128 and M == n_out_chunks * 128
    return np.ascontiguousarray(
        w.reshape(n_k_chunks, 128, n_out_chunks, 128).transpose(1, 2, 0, 3))


def _pack_rhs(w):
    # w: (D, F) -> (128, k_chunk, F//512, 512)
    K, F = w.shape
    return np.ascontiguousarray(
        w.reshape(KC, 128, F // 512, 512).transpose(1, 0, 2, 3))


def _host_inputs(x, c, encoder_out, encoder_mask, mask, cos, sin,
                 norm1_w, w_qkv, w_attn_out, adaLN_w, adaLN_b,
                 ca_norm_w, w_q, w_kv, w_o, norm2_w,
                 w_mlp1, b_mlp1, w_mlp2, b_mlp2):
    f32 = np.float32
    x = np.asarray(x, f32); c = np.asarray(c, f32)
    encoder_out = np.asarray(encoder_out, f32)
    encoder_mask = np.asarray(encoder_mask)
    mask = np.asarray(mask)
    cos = np.asarray(cos, f32); sin = np.asarray(sin, f32)
    w_qkv = np.asarray(w_qkv, f32)
    w_kv = np.asarray(w_kv, f32)

    R = np.zeros((128, 128), f32)
    for blk in range(2):
        o = 64 * blk
        for i in range(32):
            R[o + i, o + 32 + i] = -1.0
            R[o + 32 + i, o + i] = 1.0

    shared = {
        "wqk": np.concatenate(
            [_pack_lhsT(w_qkv[:, 0:D] / 8.0, KC, KC),
             _pack_lhsT(w_qkv[:, D:2 * D], KC, KC)], axis=1),
        "wv": _pack_rhs(w_qkv[:, 2 * D:3 * D]),
        "wada": np.ascontiguousarray(
            np.asarray(adaLN_w, f32).reshape(128, 48, 128)),
        "wao": _pack_lhsT(np.asarray(w_attn_out, f32), KC, KC),
        "wqca": _pack_lhsT(np.asarray(w_q, f32) / 8.0, KC, KC),
        "wkca": _pack_lhsT(w_kv[:, 0:D], KC, KC),
        "wvca": _pack_rhs(w_kv[:, D:2 * D]),
        "woca": _pack_lhsT(np.asarray(w_o, f32), KC, KC),
        "wm1": _pack_lhsT(np.asarray(w_mlp1, f32), MC, KC),
        "wm2": _pack_lhsT(np.asarray(w_mlp2, f32), KC, MC),
        "rmat": np.ascontiguousarray(R.T),
        "iden": np.eye(128, dtype=f32),
        "onec": np.ones((128, 1), f32),
        "oner": np.ones((1, 128), f32),
        "maskT": np.ascontiguousarray(
            np.where(mask, 0.0, NEG).astype(f32).T.reshape(2, 128, T)
            .transpose(1, 0, 2)),
        "cosT": np.ascontiguousarray(np.tile(cos.T, (2, 4))),
        "sinT": np.ascontiguousarray(np.tile(sin.T, (2, 4))),
        "vecs": np.ascontiguousarray(np.stack(
            [np.asarray(norm1_w, f32).reshape(KC, 128).T,
             np.asarray(ca_norm_w, f32).reshape(KC, 128).T,
             np.asarray(norm2_w, f32).reshape(KC, 128).T], axis=-1)),
        "b1c": np.ascontiguousarray(np.asarray(b_mlp1, f32).reshape(MC, 128).T),
        "b2c": np.ascontiguousarray(np.asarray(b_mlp2, f32).reshape(KC, 128).T),
        "abc": np.ascontiguousarray(np.asarray(adaLN_b, f32).reshape(48, 128).T),
    }

    in_maps = []
    for core in range(NCORES):
        bs = slice(core * BL, (core + 1) * BL)
        m = dict(shared)
        m["xT"] = np.ascontiguousarray(x[bs].transpose(0, 2, 1))
        m["cT"] = np.ascontiguousarray(c[bs].T)
        m["encT"] = np.ascontiguousarray(encoder_out[bs].transpose(0, 2, 1))
        enc_add = np.where(encoder_mask[bs], EXP_SHIFT, NEG).astype(f32)
        m["encb"] = np.ascontiguousarray(
            enc_add.reshape(BL, 4, 128).transpose(2, 0, 1))
        in_maps.append(m)
    return in_maps


_NC_CACHE = {}


def _get_nc(debug=False):
    if debug not in _NC_CACHE:
        _NC_CACHE[debug] = build_nc(debug)
    return _NC_CACHE[debug]


def run(inputs, debug=False, trace=False, **kw):
    nc = _get_nc(debug)
    in_maps = _host_inputs(**inputs)
    br = run_bass_kernel_spmd(nc, in_maps, list(range(NCORES)), trace=trace, **kw)
    outs = [r["outT"].transpose(0, 2, 1) for r in br.results]
    return np.concatenate(outs, 0).astype(np.float32), br


def kernel(**inputs):
    out, _ = run(inputs)
    return out


if __name__ == "__main__":
    nc = build_nc()
    n = sum(len(bb.instructions) for fn in nc.m.functions for bb in fn.blocks)
    print("instructions:", n)


# revision 14
# speedup vs baseline: 1.1302x; 1.1302x over previous
"""BlockDiffusionDecoder Trainium2 kernel.

Data-parallel over batch (8 batches/core x 8 cores). Feature-major
activations; fp32r GEMMs (N=512); bf16 attention core with k-major
softmax (PE-computed denominators, gpsimd recip broadcast).

kernel(**inputs) takes FULL unsharded inputs (keyed as in
reference.setup_inputs) and returns the FULL (64, 256, 1024) output.
"""
import numpy as np
import ml_dtypes

import concourse.bass as bass
import concourse.mybir as mybir
import concourse.tile as tile
from concourse import bacc
from concourse.bass_utils import run_bass_kernel_spmd

# problem dims (hardcoded per harness contract)
B, S, D, H, CD, TV = 64, 128, 1024, 16, 128, 512
HD = D // H            # 64
MLP = 4 * D            # 4096
T = 2 * S              # 256 tokens per batch
NCORES = 8
BL = B // NCORES       # 8 batches per core
NPAIR = BL // 2        # batch pairs for wide GEMM stages
KC = D // 128          # 8 feature chunks
MC = MLP // 128        # 32 mlp chunks

F32 = mybir.dt.float32
F32R = mybir.dt.float32r
BF16 = mybir.dt.bfloat16
AF = mybir.ActivationFunctionType
ALU = mybir.AluOpType

NEG = -30000.0         # additive mask for "don't attend"
EXP_SHIFT = -8.0       # constant inside exp for range safety


def build_nc(debug=False):
    nc = bacc.Bacc()
    dbg = "ExternalOutput" if debug else "Internal"

    def param(name, shape, dt=F32R):
        return nc.declare_dram_parameter(name, list(shape), dt, isOutput=False)

    # per-core activations
    xT = param("xT", (BL, D, T))                 # x^T feature-major
    cT = param("cT", (CD, BL))
    encT = param("encT", (BL, D, TV))
    encb = param("encb", (128, BL, 4), F32)      # additive enc-mask cols (w/ shift)

    # weights, packed partition-first: [128, out_chunk, k_chunk, 128]
    wqk = param("wqk", (128, 16, KC, 128))       # q,k cols of w_qkv (q /8)
    wv = param("wv", (128, KC, 2, 512))          # v cols of w_qkv (rhs layout)
    wada = param("wada", (128, 48, 128))
    wao = param("wao", (128, KC, KC, 128))
    wqca = param("wqca", (128, KC, KC, 128))     # w_q / 8
    wkca = param("wkca", (128, KC, KC, 128))     # k cols of w_kv
    wvca = param("wvca", (128, KC, 2, 512))      # v cols of w_kv (rhs layout)
    woca = param("woca", (128, KC, KC, 128))
    wm1 = param("wm1", (128, MC, KC, 128))
    wm2 = param("wm2", (128, KC, MC, 128))

    # constants
    rmat = param("rmat", (128, 128))             # R^T for rope rotate
    idenb = param("idenb", (128, 128), BF16)
    onec = param("onec", (128, 1))               # ones col fp32r (LN stats)
    onecb = param("onecb", (128, 1), BF16)       # ones col bf16 (denoms)
    oner = param("oner", (1, 128))               # ones row fp32r (LN bcast)
    maskT = param("maskT", (128, 2, T), BF16)    # additive self mask^T tiles
    cosT = param("cosT", (128, 512), F32)
    sinT = param("sinT", (128, 512), F32)
    vecs = param("vecs", (128, KC, 3), F32)      # norm1_w / ca_norm_w / norm2_w
    b1c = param("b1c", (128, MC), F32)
    b2c = param("b2c", (128, KC), F32)
    abc = param("abc", (128, 48), F32)

    outT = nc.declare_dram_parameter("outT", [BL, D, T], F32, isOutput=True)

    # scratch
    qT = nc.dram_tensor("qT", [NPAIR, KC, 128, 512], BF16, kind=dbg)
    kT = nc.dram_tensor("kT", [NPAIR, KC, 128, 512], BF16, kind=dbg)
    vtk = nc.dram_tensor("vtk", [BL, 2, 128, D], BF16, kind=dbg)
    x2T = nc.dram_tensor("x2T", [BL, KC, 128, T], F32R, kind=dbg)
    kcT = nc.dram_tensor("kcT", [BL, KC, 128, TV], BF16, kind=dbg)
    vcT = nc.dram_tensor("vcT", [BL, 4, 128, D], BF16, kind=dbg)
    x3T = nc.dram_tensor("x3T", [BL, KC, 128, T], F32R, kind=dbg)
    gT = nc.dram_tensor("gT", [NPAIR, MC, 128, 512], F32R, kind=dbg)

    with tile.TileContext(nc) as tc:
        # ------------------------------------------------------- persistent
        pers = tc.alloc_tile_pool(name="pers", bufs=1)
        rm_sb = pers.tile_from(rmat[:])
        id_sb = pers.tile_from(idenb[:])
        onec_sb = pers.tile_from(onec[:])
        onecb_sb = pers.tile_from(onecb[:])
        oner_sb = pers.tile_from(oner[:])
        mskT_sb = pers.tile_from(maskT[:])
        cos_sb = pers.tile_from(cosT[:])
        sin_sb = pers.tile_from(sinT[:])
        vec_sb = pers.tile_from(vecs[:])
        b1_sb = pers.tile_from(b1c[:])
        b2_sb = pers.tile_from(b2c[:])
        ab_sb = pers.tile_from(abc[:])
        enb_sb = pers.tile_from(encb[:])
        ct_sb = pers.tile_from(cT[:])

        esh_sb = pers.tile([128, 1], F32)
        nc.vector.memset(esh_sb[:], EXP_SHIFT)
        # mod = c @ adaLN_w + b  (feature-major cols [128, BL] per chunk f)
        mod_sb = pers.tile([128, 48, BL], F32)
        sc1_sb = pers.tile([128, KC, BL], F32)   # (1+sc_msa)*norm1_w
        sc2_sb = pers.tile([128, KC, BL], F32)   # (1+sc_mlp)*norm2_w
        b2g_sb = pers.tile([128, KC, BL], F32)   # b_mlp2 * g_mlp
        with tc.psum_pool(name="modps", bufs=1) as mps, \
             tc.sbuf_pool(name="modsb", bufs=1) as msb:
            wada_sb = msb.tile_from(wada[:])
            for f in range(48):
                mp = mps.tile([128, BL], F32, tag="mp", bufs=2)
                nc.tensor.matmul(mp[:], wada_sb[:, f, :], ct_sb[:],
                                 start=True, stop=True)
                nc.scalar.activation(mod_sb[:, f, :], mp[:], AF.Identity,
                                     bias=ab_sb[:, f:f + 1], scale=1.0)
            for c in range(KC):
                nc.vector.tensor_scalar(sc1_sb[:, c, :], mod_sb[:, 8 + c, :],
                                        1.0, vec_sb[:, c, 0:1],
                                        ALU.add, ALU.mult)
                nc.vector.tensor_scalar(sc2_sb[:, c, :], mod_sb[:, 32 + c, :],
                                        1.0, vec_sb[:, c, 2:3],
                                        ALU.add, ALU.mult)
                nc.vector.tensor_scalar(b2g_sb[:, c, :], mod_sb[:, 40 + c, :],
                                        b2_sb[:, c:c + 1], None, ALU.mult)

        def mcol(base, c, b):
            return mod_sb[:, base + c, b:b + 1]

        # feature-major LN: src(c)/out(c) -> [128, W] APs.
        def layer_norm(sbp, psp, W, src, out, scale_fn, shift_fn, halves):
            sum_ps = psp.tile([1, W], F32, tag="lnrow", bufs=2)
            ssq_ps = psp.tile([1, W], F32, tag="lnrow", bufs=2)
            for c in range(KC):
                sq = sbp.tile([128, W], F32R, tag="lnsq", bufs=1)
                nc.scalar.square(sq[:], src(c))
                nc.tensor.matmul(sum_ps[:], onec_sb[:], src(c),
                                 start=(c == 0), stop=(c == KC - 1))
                nc.tensor.matmul(ssq_ps[:], onec_sb[:], sq[:],
                                 start=(c == 0), stop=(c == KC - 1))
            mean = sbp.tile([1, W], F32R, tag="lnmean", bufs=1)
            nc.vector.tensor_scalar(mean[:], sum_ps[:], 1.0 / D, None, ALU.mult)
            ex2 = sbp.tile([1, W], F32, tag="lnex2", bufs=1)
            nc.vector.tensor_scalar(ex2[:], ssq_ps[:], 1.0 / D, None, ALU.mult)
            var = sbp.tile([1, W], F32, tag="lnvar", bufs=1)
            nc.vector.tensor_tensor(var[:], mean[:], mean[:], ALU.mult)
            nc.vector.tensor_tensor(var[:], ex2[:], var[:], ALU.subtract)
            nc.vector.tensor_scalar(var[:], var[:], 1e-5, None, ALU.add)
            std = sbp.tile([1, W], F32, tag="lnstd", bufs=1)
            nc.scalar.activation(std[:], var[:], AF.Sqrt, bias=0.0, scale=1.0)
            rs = sbp.tile([1, W], F32R, tag="lnrs", bufs=1)
            with nc.allow_low_precision(reason="fp32r rounding ok"):
                nc.vector.reciprocal(rs[:], std[:])
            mb_ps = psp.tile([128, W], F32, tag="lnbc", bufs=2)
            rs_ps = psp.tile([128, W], F32, tag="lnbc", bufs=2)
            nc.tensor.matmul(mb_ps[:], oner_sb[:], mean[:], start=True, stop=True)
            nc.tensor.matmul(rs_ps[:], oner_sb[:], rs[:], start=True, stop=True)
            for c in range(KC):
                t = sbp.tile([128, W], F32, tag="lnt", bufs=1)
                nc.vector.tensor_tensor(t[:], src(c), mb_ps[:], ALU.subtract)
                t2 = sbp.tile([128, W], F32, tag="lnt2", bufs=1)
                nc.vector.tensor_tensor(t2[:], t[:], rs_ps[:], ALU.mult)
                o = out(c)
                for hi, (lo, hi_) in enumerate(halves):
                    nc.scalar.activation(
                        o[:, lo:hi_], t2[:, lo:hi_], AF.Identity,
                        bias=shift_fn(c, hi), scale=scale_fn(c, hi))

        # ------------------------------------------------- stage 1: LN1+QKV
        with tc.sbuf_pool(name="s1w", bufs=1) as s1w, \
             tc.sbuf_pool(name="s1a", bufs=1) as s1a, \
             tc.sbuf_pool(name="s1t", bufs=1) as s1t, \
             tc.psum_pool(name="s1p", bufs=1) as s1p:
            wqk_sb = s1w.tile_from(wqk[:])
            wv_sb = s1w.tile_from(wv[:])
            for p in range(NPAIR):
                b0 = 2 * p
                x_sb = s1a.tile([128, KC, 512], F32R, tag="x", bufs=1)
                for c in range(KC):
                    nc.sync.dma_start(x_sb[:, c, 0:256],
                                      xT[b0, 128 * c:128 * (c + 1), :])
                    nc.sync.dma_start(x_sb[:, c, 256:512],
                                      xT[b0 + 1, 128 * c:128 * (c + 1), :])
                xn_sb = s1a.tile([128, KC, 512], F32R, tag="xn", bufs=1)
                layer_norm(
                    s1t, s1p, 512,
                    src=lambda c: x_sb[:, c, :],
                    out=lambda c: xn_sb[:, c, :],
                    scale_fn=lambda c, hi: sc1_sb[:, c, b0 + hi:b0 + hi + 1],
                    shift_fn=lambda c, hi: mcol(0, c, b0 + hi),
                    halves=[(0, 256), (256, 512)])

                # q, k feature-major chunks + rope (bf16 out)
                for f in range(16):
                    dst = qT if f < 8 else kT
                    qp = s1p.tile([128, 512], F32, tag="gemm", bufs=2)
                    for c in range(KC):
                        nc.tensor.matmul(qp[:], wqk_sb[:, f, c, :], xn_sb[:, c, :],
                                         start=(c == 0), stop=(c == KC - 1))
                    q_sb = s1t.tile([128, 512], F32R, tag="q", bufs=2)
                    nc.scalar.copy(q_sb[:], qp[:])
                    rp = s1p.tile([128, 512], F32, tag="rot", bufs=2)
                    nc.tensor.matmul(rp[:], rm_sb[:], q_sb[:], start=True, stop=True)
                    m1 = s1t.tile([128, 512], F32, tag="m1", bufs=2)
                    nc.vector.tensor_tensor(m1[:], q_sb[:], cos_sb[:], ALU.mult)
                    qr = s1t.tile([128, 512], BF16, tag="qr", bufs=2)
                    nc.vector.tensor_tensor(qr[:], rp[:], sin_sb[:], ALU.mult)
                    nc.vector.tensor_tensor(qr[:], m1[:], qr[:], ALU.add)
                    nc.sync.dma_start(dst[p, f % 8], qr[:])

                # v token-major (bf16 out)
                for hi in range(2):
                    for tt in range(2):
                        for fc in range(2):
                            vp = s1p.tile([128, 512], F32, tag="gemm", bufs=2)
                            lo = 256 * hi + 128 * tt
                            for c in range(KC):
                                nc.tensor.matmul(vp[:], xn_sb[:, c, lo:lo + 128],
                                                 wv_sb[:, c, fc, :],
                                                 start=(c == 0), stop=(c == KC - 1))
                            v_sb = s1t.tile([128, 512], BF16, tag="v", bufs=2)
                            nc.scalar.copy(v_sb[:], vp[:])
                            nc.sync.dma_start(
                                vtk[b0 + hi, tt, :, 512 * fc:512 * (fc + 1)],
                                v_sb[:])

        # --------------------------------------- stage 2: self-attn + proj
        with tc.sbuf_pool(name="s2w", bufs=1) as s2w, \
             tc.sbuf_pool(name="s2a", bufs=1) as s2a, \
             tc.sbuf_pool(name="s2t", bufs=1) as s2t, \
             tc.psum_pool(name="s2p", bufs=1) as s2p:
            wao_sb = s2w.tile_from(wao[:])
            for p in range(NPAIR):
                b0 = 2 * p
                a2_sb = s2a.tile([128, KC, 512], F32R, tag="a2", bufs=2)
                xp_sb = s2a.tile([128, KC, 512], F32R, tag="xp", bufs=2)
                for hf in range(2):
                    b = b0 + hf
                    qb = s2a.tile([128, KC, 256], BF16, tag="qb", bufs=2)
                    kb = s2a.tile([128, KC, 256], BF16, tag="kb", bufs=2)
                    vb = s2a.tile([128, 2, D], BF16, tag="vb", bufs=2)
                    for c in range(KC):
                        nc.sync.dma_start(qb[:, c, :],
                                          qT[p, c, :, 256 * hf:256 * (hf + 1)])
                        nc.sync.dma_start(kb[:, c, :],
                                          kT[p, c, :, 256 * hf:256 * (hf + 1)])
                        nc.sync.dma_start(xp_sb[:, c, 256 * hf:256 * (hf + 1)],
                                          xT[b, 128 * c:128 * (c + 1), :])
                    for kt in range(2):
                        nc.sync.dma_start(vb[:, kt, :], vtk[b, kt])
                    for j in range(KC):   # head pairs
                        pt = s2t.tile([128, 2, 2, 256], BF16, tag="pt", bufs=2)
                        for kt in range(2):
                            for hh in range(2):
                                hr = slice(64 * hh, 64 * hh + 64)
                                sp = s2p.tile([128, 256], F32, tag="sp", bufs=2)
                                nc.tensor.matmul(
                                    sp[:], kb[hr, j, 128 * kt:128 * (kt + 1)],
                                    qb[hr, j, :], start=True, stop=False)
                                nc.tensor.matmul(sp[:], id_sb[:], mskT_sb[:, kt, :],
                                                 start=False, stop=True)
                                nc.scalar.activation(pt[:, kt, hh, :], sp[:],
                                                     AF.Exp, bias=esh_sb[:],
                                                     scale=1.0)
                        dn_ps = s2p.tile([1, 512], F32, tag="dn", bufs=1)
                        for kt in range(2):
                            nc.tensor.matmul(
                                dn_ps[:], onecb_sb[:],
                                pt[:, kt, :, :].rearrange("p a b -> p (a b)"),
                                start=(kt == 0), stop=(kt == 1))
                        rc_sb = s2t.tile([1, 512], F32, tag="rc", bufs=2)
                        nc.vector.reciprocal(rc_sb[:], dn_ps[:])
                        rb_sb = s2t.tile([64, 512], F32, tag="rb", bufs=2)
                        nc.gpsimd.partition_broadcast(rb_sb[:], rc_sb[:])
                        av_ps = s2p.tile([128, 256], F32, tag="av", bufs=2)
                        for hh in range(2):
                            for kt in range(2):
                                nc.tensor.matmul(
                                    av_ps[64 * hh:64 * hh + 64, :],
                                    vb[:, kt,
                                       128 * j + 64 * hh:128 * j + 64 * hh + 64],
                                    pt[:, kt, hh, :],
                                    start=(kt == 0), stop=(kt == 1),
                                    tile_position=(0, 64 * hh))
                        for hh in range(2):
                            hr = slice(64 * hh, 64 * hh + 64)
                            nc.vector.tensor_tensor(
                                a2_sb[hr, j, 256 * hf:256 * (hf + 1)],
                                av_ps[hr, :],
                                rb_sb[0:64, 256 * hh:256 * hh + 256], ALU.mult)
                # out proj + gated residual -> x2 (pair, N=512)
                for oc in range(KC):
                    op = s2p.tile([128, 512], F32, tag="op", bufs=2)
                    for c in range(KC):
                        nc.tensor.matmul(op[:], wao_sb[:, oc, c, :], a2_sb[:, c, :],
                                         start=(c == 0), stop=(c == KC - 1))
                    for hf in range(2):
                        sl = slice(256 * hf, 256 * (hf + 1))
                        x2_sb = s2t.tile([128, 256], F32R, tag="x2", bufs=2)
                        nc.vector.scalar_tensor_tensor(
                            x2_sb[:], op[:, sl], mcol(16, oc, b0 + hf),
                            xp_sb[:, oc, sl], ALU.mult, ALU.add)
                        nc.sync.dma_start(x2T[b0 + hf, oc], x2_sb[:])

        # ------------------------------------------------ stage 3: CA k/v
        with tc.sbuf_pool(name="s3w", bufs=1) as s3w, \
             tc.sbuf_pool(name="s3a", bufs=1) as s3a, \
             tc.sbuf_pool(name="s3t", bufs=1) as s3t, \
             tc.psum_pool(name="s3p", bufs=1) as s3p:
            wk_sb = s3w.tile_from(wkca[:])
            wvc_sb = s3w.tile_from(wvca[:])
            for b in range(BL):
                e_sb = s3a.tile([128, KC, TV], F32R, tag="e", bufs=2)
                for c in range(KC):
                    nc.sync.dma_start(e_sb[:, c, :], encT[b, 128 * c:128 * (c + 1), :])
                for f in range(KC):
                    kp = s3p.tile([128, TV], F32, tag="gemm", bufs=2)
                    for c in range(KC):
                        nc.tensor.matmul(kp[:], wk_sb[:, f, c, :], e_sb[:, c, :],
                                         start=(c == 0), stop=(c == KC - 1))
                    kc_sb = s3t.tile([128, TV], BF16, tag="kc", bufs=2)
                    nc.scalar.copy(kc_sb[:], kp[:])
                    nc.sync.dma_start(kcT[b, f], kc_sb[:])
                for tt in range(4):
                    for fcc in range(2):
                        vp = s3p.tile([128, 512], F32, tag="gemm", bufs=2)
                        for c in range(KC):
                            nc.tensor.matmul(vp[:],
                                             e_sb[:, c, 128 * tt:128 * (tt + 1)],
                                             wvc_sb[:, c, fcc, :],
                                             start=(c == 0), stop=(c == KC - 1))
                        vc_sb = s3t.tile([128, 512], BF16, tag="vc", bufs=2)
                        nc.scalar.copy(vc_sb[:], vp[:])
                        nc.sync.dma_start(vcT[b, tt, :, 512 * fcc:512 * (fcc + 1)],
                                          vc_sb[:])

        # ------------------------------------- stage 4: CA attn + proj
        with tc.sbuf_pool(name="s4w", bufs=1) as s4w, \
             tc.sbuf_pool(name="s4a", bufs=1) as s4a, \
             tc.sbuf_pool(name="s4t", bufs=1) as s4t, \
             tc.psum_pool(name="s4p", bufs=1) as s4p:
            wq_sb = s4w.tile_from(wqca[:])
            wo_sb = s4w.tile_from(woca[:])
            for p in range(NPAIR):
                b0 = 2 * p
                x2_sb = s4a.tile([128, KC, 512], F32R, tag="x2b", bufs=2)
                for c in range(KC):
                    nc.sync.dma_start(x2_sb[:, c, 0:256], x2T[b0, c])
                    nc.sync.dma_start(x2_sb[:, c, 256:512], x2T[b0 + 1, c])
                xc_sb = s4a.tile([128, KC, 512], F32R, tag="xc", bufs=1)
                layer_norm(
                    s4t, s4p, 512,
                    src=lambda c: x2_sb[:, c, :],
                    out=lambda c: xc_sb[:, c, :],
                    scale_fn=lambda c, hi: vec_sb[:, c, 1:2],
                    shift_fn=lambda c, hi: 0.0,
                    halves=[(0, 512)])
                qc_sb = s4a.tile([128, KC, 512], BF16, tag="qc", bufs=1)
                for f in range(KC):
                    qp = s4p.tile([128, 512], F32, tag="lnbc", bufs=2)
                    for c in range(KC):
                        nc.tensor.matmul(qp[:], wq_sb[:, f, c, :], xc_sb[:, c, :],
                                         start=(c == 0), stop=(c == KC - 1))
                    nc.scalar.copy(qc_sb[:, f, :], qp[:])
                a2_sb = s4a.tile([128, KC, 512], F32R, tag="ca2", bufs=1)
                for hf in range(2):
                    b = b0 + hf
                    kc_sb = s4a.tile([128, KC, TV], BF16, tag="kcb", bufs=2)
                    vc_sb = s4a.tile([128, 4, D], BF16, tag="vcb", bufs=2)
                    for c in range(KC):
                        nc.sync.dma_start(kc_sb[:, c, :], kcT[b, c])
                    for kt in range(4):
                        nc.sync.dma_start(vc_sb[:, kt, :], vcT[b, kt])
                    qsl = slice(256 * hf, 256 * (hf + 1))
                    for j in range(KC):
                        pt = s4t.tile([128, 4, 2, 256], BF16, tag="cpt", bufs=1)
                        for kt in range(4):
                            for hh in range(2):
                                hr = slice(64 * hh, 64 * hh + 64)
                                sp = s4p.tile([128, 256], F32, tag="sp", bufs=2)
                                nc.tensor.matmul(
                                    sp[:], kc_sb[hr, j, 128 * kt:128 * (kt + 1)],
                                    qc_sb[hr, j, qsl], start=True, stop=True)
                                nc.scalar.activation(pt[:, kt, hh, :], sp[:],
                                                     AF.Exp,
                                                     bias=enb_sb[:, b, kt:kt + 1],
                                                     scale=1.0)
                        dn_ps = s4p.tile([1, 512], F32, tag="lnrow", bufs=2)
                        for kt in range(4):
                            nc.tensor.matmul(
                                dn_ps[:], onecb_sb[:],
                                pt[:, kt, :, :].rearrange("p a b -> p (a b)"),
                                start=(kt == 0), stop=(kt == 3))
                        rc_sb = s4t.tile([1, 512], F32, tag="crc", bufs=1)
                        nc.vector.reciprocal(rc_sb[:], dn_ps[:])
                        rb_sb = s4t.tile([64, 512], F32, tag="crb", bufs=1)
                        nc.gpsimd.partition_broadcast(rb_sb[:], rc_sb[:])
                        av_ps = s4p.tile([128, 256], F32, tag="av", bufs=2)
                        for hh in range(2):
                            for kt in range(4):
                                nc.tensor.matmul(
                                    av_ps[64 * hh:64 * hh + 64, :],
                                    vc_sb[:, kt,
                                          128 * j + 64 * hh:128 * j + 64 * hh + 64],
                                    pt[:, kt, hh, :],
                                    start=(kt == 0), stop=(kt == 3),
                                    tile_position=(0, 64 * hh))
                        for hh in range(2):
                            hr = slice(64 * hh, 64 * hh + 64)
                            nc.vector.tensor_tensor(
                                a2_sb[hr, j, qsl], av_ps[hr, :],
                                rb_sb[0:64, 256 * hh:256 * hh + 256], ALU.mult)
                for oc in range(KC):
                    op = s4p.tile([128, 512], F32, tag="lnbc", bufs=2)
                    for c in range(KC):
                        nc.tensor.matmul(op[:], wo_sb[:, oc, c, :], a2_sb[:, c, :],
                                         start=(c == 0), stop=(c == KC - 1))
                    x3_sb = s4t.tile([128, 512], F32R, tag="x3", bufs=2)
                    nc.vector.tensor_tensor(x3_sb[:], op[:], x2_sb[:, oc, :],
                                            ALU.add)
                    nc.sync.dma_start(x3T[b0, oc], x3_sb[:, 0:256])
                    nc.sync.dma_start(x3T[b0 + 1, oc], x3_sb[:, 256:512])

        # ---------------------------------------------- stage 5: LN2+MLP1
        with tc.sbuf_pool(name="s5h", bufs=1) as s5h, \
             tc.sbuf_pool(name="s5w", bufs=1) as s5w, \
             tc.sbuf_pool(name="s5t", bufs=1) as s5t, \
             tc.psum_pool(name="s5p", bufs=1) as s5p:
            h_sb = s5h.tile([128, NPAIR, KC, 512], F32R)
            for p in range(NPAIR):
                b0 = 2 * p
                x3_sb = s5t.tile([128, KC, 512], F32R, tag="x3b", bufs=2)
                for c in range(KC):
                    nc.sync.dma_start(x3_sb[:, c, 0:256], x3T[b0, c])
                    nc.sync.dma_start(x3_sb[:, c, 256:512], x3T[b0 + 1, c])
                layer_norm(
                    s5t, s5p, 512,
                    src=lambda c: x3_sb[:, c, :],
                    out=lambda c: h_sb[:, p, c, :],
                    scale_fn=lambda c, hi: sc2_sb[:, c, b0 + hi:b0 + hi + 1],
                    shift_fn=lambda c, hi: mcol(24, c, b0 + hi),
                    halves=[(0, 256), (256, 512)])
            for mc in range(MC):
                wt = s5w.tile([128, KC, 128], F32R, tag="w1", bufs=3)
                nc.sync.dma_start(wt[:], wm1[:, mc])
                for p in range(NPAIR):
                    gp = s5p.tile([128, 512], F32, tag="lnbc", bufs=2)
                    for c in range(KC):
                        nc.tensor.matmul(gp[:], wt[:, c, :], h_sb[:, p, c, :],
                                         start=(c == 0), stop=(c == KC - 1))
                    g_sb = s5t.tile([128, 512], F32R, tag="g", bufs=3)
                    nc.scalar.activation(g_sb[:], gp[:], AF.Gelu_apprx_tanh,
                                         bias=b1_sb[:, mc:mc + 1], scale=1.0)
                    nc.sync.dma_start(gT[p, mc], g_sb[:])

        # ------------------------------------- stage 6: MLP2 + residual
        with tc.sbuf_pool(name="s6w", bufs=1) as s6w, \
             tc.sbuf_pool(name="s6a", bufs=1) as s6a, \
             tc.sbuf_pool(name="s6t", bufs=1) as s6t, \
             tc.psum_pool(name="s6p", bufs=1) as s6p:
            wm2_sb = s6w.tile_from(wm2[:])
            for p in range(NPAIR):
                b0 = 2 * p
                x3_sb = s6a.tile([128, KC, 512], F32R, tag="x3c", bufs=2)
                for c in range(KC):
                    nc.sync.dma_start(x3_sb[:, c, 0:256], x3T[b0, c])
                    nc.sync.dma_start(x3_sb[:, c, 256:512], x3T[b0 + 1, c])
                mp = [s6p.tile([128, 512], F32, tag=f"mp{oc}", bufs=1,
                               name=f"mp{oc}") for oc in range(KC)]
                for kcc in range(MC):
                    g_sb = s6a.tile([128, 512], F32R, tag="gin", bufs=3)
                    nc.sync.dma_start(g_sb[:], gT[p, kcc])
                    for oc in range(KC):
                        nc.tensor.matmul(mp[oc][:], wm2_sb[:, oc, kcc, :], g_sb[:],
                                         start=(kcc == 0), stop=(kcc == MC - 1))
                for oc in range(KC):
                    for hi in range(2):
                        bb = b0 + hi
                        sl = slice(256 * hi, 256 * (hi + 1))
                        gm = s6t.tile([128, 256], F32, tag="gm", bufs=2)
                        nc.scalar.activation(gm[:], mp[oc][:, sl], AF.Identity,
                                             bias=b2g_sb[:, oc, bb:bb + 1],
                                             scale=mcol(40, oc, bb))
                        o_sb = s6t.tile([128, 256], F32, tag="o", bufs=2)
                        nc.vector.tensor_tensor(o_sb[:], gm[:], x3_sb[:, oc, sl],
                                                ALU.add)
                        nc.sync.dma_start(outT[bb, 128 * oc:128 * (oc + 1), :],
                                          o_sb[:])

        pers.release()

    nc.compile()
    return nc


# ---------------------------------------------------------------------------
# host side
# ---------------------------------------------------------------------------

def _pack_lhsT(w, n_out_chunks, n_k_chunks):
    K, M = w.shape
    assert K == n_k_chunks * 128 and M == n_out_chunks * 128
    return np.ascontiguousarray(
        w.reshape(n_k_chunks, 128, n_out_chunks, 128).transpose(1, 2, 0, 3))


def _pack_rhs(w):
    K, F = w.shape
    return np.ascontiguousarray(
        w.reshape(KC, 128, F // 512, 512).transpose(1, 0, 2, 3))


def _host_inputs(x, c, encoder_out, encoder_mask, mask, cos, sin,
                 norm1_w, w_qkv, w_attn_out, adaLN_w, adaLN_b,
                 ca_norm_w, w_q, w_kv, w_o, norm2_w,
                 w_mlp1, b_mlp1, w_mlp2, b_mlp2):
    f32 = np.float32
    bf16 = ml_dtypes.bfloat16
    x = np.asarray(x, f32); c = np.asarray(c, f32)
    encoder_out = np.asarray(encoder_out, f32)
    encoder_mask = np.asarray(encoder_mask)
    mask = np.asarray(mask)
    cos = np.asarray(cos, f32); sin = np.asarray(sin, f32)
    w_qkv = np.asarray(w_qkv, f32)
    w_kv = np.asarray(w_kv, f32)

    R = np.zeros((128, 128), f32)
    for blk in range(2):
        o = 64 * blk
        for i in range(32):
            R[o + i, o + 32 + i] = -1.0
            R[o + 32 + i, o + i] = 1.0

    shared = {
        "wqk": np.concatenate(
            [_pack_lhsT(w_qkv[:, 0:D] / 8.0, KC, KC),
             _pack_lhsT(w_qkv[:, D:2 * D], KC, KC)], axis=1),
        "wv": _pack_rhs(w_qkv[:, 2 * D:3 * D]),
        "wada": np.ascontiguousarray(
            np.asarray(adaLN_w, f32).reshape(128, 48, 128)),
        "wao": _pack_lhsT(np.asarray(w_attn_out, f32), KC, KC),
        "wqca": _pack_lhsT(np.asarray(w_q, f32) / 8.0, KC, KC),
        "wkca": _pack_lhsT(w_kv[:, 0:D], KC, KC),
        "wvca": _pack_rhs(w_kv[:, D:2 * D]),
        "woca": _pack_lhsT(np.asarray(w_o, f32), KC, KC),
        "wm1": _pack_lhsT(np.asarray(w_mlp1, f32), MC, KC),
        "wm2": _pack_lhsT(np.asarray(w_mlp2, f32), KC, MC),
        "rmat": np.ascontiguousarray(R.T),
        "idenb": np.eye(128, dtype=bf16),
        "onec": np.ones((128, 1), f32),
        "onecb": np.ones((128, 1), bf16),
        "oner": np.ones((1, 128), f32),
        "maskT": np.ascontiguousarray(
            np.where(mask, 0.0, NEG).astype(f32).T.reshape(2, 128, T)
            .transpose(1, 0, 2)).astype(bf16),
        "cosT": np.ascontiguousarray(np.tile(cos.T, (2, 4))),
        "sinT": np.ascontiguousarray(np.tile(sin.T, (2, 4))),
        "vecs": np.ascontiguousarray(np.stack(
            [np.asarray(norm1_w, f32).reshape(KC, 128).T,
             np.asarray(ca_norm_w, f32).reshape(KC, 128).T,
             np.asarray(norm2_w, f32).reshape(KC, 128).T], axis=-1)),
        "b1c": np.ascontiguousarray(np.asarray(b_mlp1, f32).reshape(MC, 128).T),
        "b2c": np.ascontiguousarray(np.asarray(b_mlp2, f32).reshape(KC, 128).T),
        "abc": np.ascontiguousarray(np.asarray(adaLN_b, f32).reshape(48, 128).T),
    }

    in_maps = []
    for core in range(NCORES):
        bs = slice(core * BL, (core + 1) * BL)
        m = dict(shared)
        m["xT"] = np.ascontiguousarray(x[bs].transpose(0, 2, 1))
        m["cT"] = np.ascontiguousarray(c[bs].T)
        m["encT"] = np.ascontiguousarray(encoder_out[bs].transpose(0, 2, 1))
        enc_add = np.where(encoder_mask[bs], EXP_SHIFT, NEG).astype(f32)
        m["encb"] = np.ascontiguousarray(
            enc_add.reshape(BL, 4, 128).transpose(2, 0, 1))
        in_maps.append(m)
    return in_maps


_NC_CACHE = {}


def _get_nc(debug=False):
    if debug not in _NC_CACHE:
        _NC_CACHE[debug] = build_nc(debug)
    return _NC_CACHE[debug]


def run(inputs, debug=False, trace=False, **kw):
    nc = _get_nc(debug)
    in_maps = _host_inputs(**inputs)
    br = run_bass_kernel_spmd(nc, in_maps, list(range(NCORES)), trace=trace, **kw)
    outs = [r["outT"].transpose(0, 2, 1) for r in br.results]
    return np.concatenate(outs, 0).astype(np.float32), br


def kernel(**inputs):
    out, _ = run(inputs)
    return out


if __name__ == "__main__":
    nc = build_nc()
    n = sum(len(bb.instructions) for fn in nc.m.functions for bb in fn.blocks)
    print("instructions:", n)
